# revision 8
# baseline (speedup 1.0000x reference)
"""Trainium2 Bass kernel for nn_BlockAttention (block-local attention with RoPE + gate).

Sharding: sequence-parallel over 8 cores. Flattened [B*S=8192, E] rows split into
8 contiguous shards of 1024 rows (4 blocks of 256; blocks never cross cores or
batch boundaries since 4096/256=16 blocks per batch, 4 per core).

Per-core layout strategy (features-on-partitions, "transposed" activations):
  - host pre-transposes the x shard to xT [E, R] so no on-chip transposes needed
  - qT/kT/gateT [E, R] = W.T @ x via matmul(lhsT=W_chunk, rhs=xT_chunk)
  - v [R, E] natural via matmul(lhsT=xT_chunk, rhs=Wv_chunk)
  - all matmul operands fp16/bf16: fp32r streams at half rate on HW and
    serializes an internal 4-byte weight load; fp16/bf16 get FWL + the
    separate LDWEIGHTS overlaps under the PE reorder window
  - RoPE applied on transposed q/k with host-prepared fp16 cos/sin tables
    (replicated per head-pair, rotate-sign folded into sin table); all DVE
    ops 16-bit for the 2x DVE rate
  - block-local attention per (block, head) with transposed scores S_T[k,q]:
    exp on ScalarE -> bf16 (bf16 range covers e^15; no max subtraction
    needed), AV via matmul(lhsT=v_block, rhs=expS_T) in bf16 with a
    trailing ones column per head so row 64 of the AV psum is the softmax
    row-sum (free), normalize via one K=1 ones-outer-product replicate
    matmul (bf16) + DVE muls
  - gate: tanh on ScalarE (same ACT table set as exp -- sigmoid would
    force a ~2.7us table reload per switch), sigmoid(g) = 0.5 + 0.5*tanh(g/2)
    finished by one fused DVE tensor_scalar; multiply fused into the
    attention tail
  - out projection back through Wo in transposed layout; host un-transposes
"""
import sys

sys.path.insert(0, "/opt/trn_rl_repo")
import numpy as np

B, S, E = 2, 4096, 1024
H, D = 16, 64
BLK = 256
NCORES = 8
R = (B * S) // NCORES   # 1024 rows per core
NB = R // BLK           # 4 blocks per core
NCH = E // 128          # 8 feature chunks of 128
SCALE = 1.0 / np.sqrt(D)


def emit(tc, outs, ins):
    """Emit the per-core program. ins/outs are DRAM APs:
    ins  = [xT, wq, wk, wv, wg, wo, cos2, sin2]
    outs = [outT]
    """
    from contextlib import ExitStack
    import concourse.mybir as mybir

    F32 = mybir.dt.float32
    F16 = mybir.dt.float16
    BF16 = mybir.dt.bfloat16
    AF = mybir.ActivationFunctionType
    ALU = mybir.AluOpType

    nc = tc.nc
    xT_d, wq_d, wk_d, wv_d, wg_d, wo_d, c2_d, s2_d = ins
    (outT_d,) = outs

    with ExitStack() as ctx:
        ep = ctx.enter_context
        consts = ep(tc.tile_pool(name="consts", bufs=1))
        big = ep(tc.tile_pool(name="big", bufs=1))
        wpool = ep(tc.tile_pool(name="wpool", bufs=3))
        wvpool = ep(tc.tile_pool(name="wvpool", bufs=2))
        ropet = ep(tc.tile_pool(name="ropet", bufs=2))
        rawp = ep(tc.tile_pool(name="rawp", bufs=2))
        espool = ep(tc.tile_pool(name="espool", bufs=4))
        smalls = ep(tc.tile_pool(name="smalls", bufs=2))
        rrsbp = ep(tc.tile_pool(name="rrsbp", bufs=2))
        ytp = ep(tc.tile_pool(name="ytp", bufs=2))
        opool = ep(tc.tile_pool(name="opool", bufs=2))
        # PSUM: 8 banks total, everything double-buffered.
        big_ps = ep(tc.tile_pool(name="big_ps", bufs=3, space="PSUM"))
        s_ps_p = ep(tc.tile_pool(name="s_ps_p", bufs=2, space="PSUM"))
        av_ps_p = ep(tc.tile_pool(name="av_ps_p", bufs=2, space="PSUM"))
        rr_ps_p = ep(tc.tile_pool(name="rr_ps_p", bufs=1, space="PSUM"))

        # ---- constants / inputs resident in SBUF
        # xt loaded by row-halves so the first projection group (which
        # streams rows 0:512 of every kc chunk) can start after ~half the
        # x bytes land.
        xt = big.tile([128, NCH, R], F16)
        for kc in range(NCH):
            nc.sync.dma_start(xt[:, kc, 0:512], xT_d[kc * 128:(kc + 1) * 128, 0:512])
        for kc in range(NCH):
            nc.sync.dma_start(xt[:, kc, 512:R], xT_d[kc * 128:(kc + 1) * 128, 512:R])
        c2 = consts.tile([128, R], F16)
        nc.sync.dma_start(c2[:], c2_d[:])
        s2 = consts.tile([128, R], F16)
        nc.sync.dma_start(s2[:], s2_d[:])
        onesrowf = consts.tile([1, 64], F32)
        nc.vector.memset(onesrowf[:], 1.0)
        onesrow = consts.tile([1, 64], BF16)
        nc.vector.tensor_copy(onesrow[:], onesrowf[:])

        qT = big.tile([128, NCH, R], F16)
        kT = big.tile([128, NCH, R], F16)
        # v holds 16 heads x (64 dims + a ones column) per row-chunk: the
        # ones column makes each AV matmul also emit the softmax row-sums
        # (output row 64) for free.
        v = big.tile([128, NCH, H * 65], BF16)
        ones16f = consts.tile([128, 16], F32)
        nc.vector.memset(ones16f[:], 1.0)
        # sg doubles as y: the attention tail multiplies the gate in-place
        # (av*rr*sg), and the out projection consumes it. fp16: it feeds
        # the fp16 out-projection matmul.
        sg = big.tile([128, NCH, R], F16)

        # ---- one projection output chunk: 8-matmul psum group + drain
        def mm_group(w, nh):
            ps = big_ps.tile([128, 512], F32, tag="big")
            for kc in range(NCH):
                nc.tensor.matmul(
                    ps[:],
                    w[:, kc, :],
                    xt[:, kc, nh * 512:(nh + 1) * 512],
                    start=(kc == 0),
                    stop=(kc == NCH - 1),
                )
            return ps

        def proj_chunk_gate(w, mc):
            # raw-gate tanh(g/2); finished to sigmoid by one fused DVE op
            for nh in range(2):
                ps = mm_group(w, nh)
                dstsl = sg[:, mc, nh * 512:(nh + 1) * 512]
                nc.scalar.activation(dstsl, ps[:], AF.Tanh, scale=0.5)
            # sigmoid(g) = 0.5*tanh(g/2) + 0.5, fp16 in-place at 2x DVE rate
            nc.vector.tensor_scalar(
                sg[:, mc, :], sg[:, mc, :], 0.5, 0.5, ALU.mult, ALU.add)

        def proj_chunk_rope(w, dst, mc):
            # RoPE: the rotate (partition swap d <-> d+-32 within each head)
            # rides on the otherwise-idle DMA engines as 4 partition-shifted
            # SBUF->SBUF copies of the raw projection; sign lives in sin2.
            ps0 = mm_group(w, 0)
            ps1 = mm_group(w, 1)
            raw = rawp.tile([128, R], F16, tag="raw")
            nc.scalar.activation(raw[:, 0:512], ps0[:], AF.Copy)
            nc.scalar.activation(raw[:, 512:1024], ps1[:], AF.Copy)
            t = ropet.tile([128, R], F16, tag="t")
            for h2 in (0, 64):
                nc.sync.dma_start(t[h2:h2 + 32, :], raw[h2 + 32:h2 + 64, :])
                nc.gpsimd.dma_start(t[h2 + 32:h2 + 64, :], raw[h2:h2 + 32, :])
            dsl = dst[:, mc, :]
            nc.vector.tensor_mul(dsl, raw[:], c2[:])
            nc.vector.tensor_mul(t[:], t[:], s2[:])
            nc.vector.tensor_add(dsl, dsl, t[:])

        def proj_load_w(w_d, mc):
            w = wpool.tile([128, NCH, 128], F16, tag="w")
            src = w_d.rearrange("(kc p) m -> p kc m", p=128)
            nc.sync.dma_start(w[:], src[:, :, mc * 128:(mc + 1) * 128])
            return w

        def attn_front(b, c):
            est = []
            for hi in range(2):
                pb = 64 * hi
                sps = s_ps_p.tile([128, 512], F32, tag="s")
                for kph in range(2):
                    nc.tensor.matmul(
                        sps[:, kph * 256:(kph + 1) * 256],
                        kT[pb:pb + 64, c,
                           b * 256 + kph * 128:b * 256 + (kph + 1) * 128],
                        qT[pb:pb + 64, c, b * 256:(b + 1) * 256],
                        start=True, stop=True,
                    )
                es = espool.tile([128, 512], BF16, tag="es")
                nc.scalar.activation(es[:], sps[:], AF.Exp,
                                     scale=float(SCALE))
                est.append(es)
            return (b, c, est)

        def attn_mid(st):
            b, c, est = st
            # both heads' AV share one psum bank: hi=0 in cols 0:256,
            # hi=1 in cols 256:512; row 0 = softmax row-sums (ones-led V)
            av = av_ps_p.tile([65, 512], F32, tag="av")
            for hi in range(2):
                es = est[hi]
                h = 2 * c + hi
                for kph in range(2):
                    nc.tensor.matmul(
                        av[:, hi * 256:(hi + 1) * 256],
                        v[:, 2 * b + kph, h * 65:(h + 1) * 65],
                        es[:, kph * 256:(kph + 1) * 256],
                        start=(kph == 0), stop=(kph == 1),
                    )
            # reciprocal_approx_fast misreads PSUM at base partition 64
            # on HW, so stage the sums row through SBUF.
            sumst = smalls.tile([1, 512], F32, tag="sumst")
            nc.vector.tensor_copy(sumst[:], av[64:65, :])
            recipf = smalls.tile([1, 512], F32, tag="recipf")
            nc.vector.reciprocal_approx_fast(recipf[:], sumst[:])
            recip = smalls.tile([1, 512], BF16, tag="recip")
            nc.scalar.activation(recip[:], recipf[:], AF.Copy)
            return (b, c, av, recip)

        def attn_tail(st):
            b, c, av, recip = st
            rr = rr_ps_p.tile([64, 512], F32, tag="rr")
            nc.tensor.matmul(rr[:], onesrow[:], recip[0:1, :],
                             start=True, stop=True)
            rrsb = rrsbp.tile([128, 256], F32, tag="rrsb")
            nc.scalar.activation(rrsb[0:64, :], rr[0:64, 0:256], AF.Copy)
            nc.scalar.activation(rrsb[64:128, :], rr[0:64, 256:512], AF.Copy)
            # y = av * rr * sg, written in-place over sg
            yt = ytp.tile([128, 256], F16, tag="yt")
            for hi in range(2):
                ysl_p = slice(64 * hi, 64 * hi + 64)
                nc.vector.tensor_mul(yt[ysl_p, :],
                                     av[0:64, hi * 256:(hi + 1) * 256],
                                     rrsb[ysl_p, :])
                ysl = sg[ysl_p, c, b * 256:(b + 1) * 256]
                nc.vector.tensor_mul(ysl, ysl, yt[ysl_p, :])

        for rc in range(NCH):
            vh = v[:, rc, :].rearrange("p (h t) -> p h t", t=65)
            nc.scalar.activation(vh[:, :, 64], ones16f[:], AF.Copy)

        # ---- fused main loop: per chunk c, emit the q/qrot/k/krot
        # projections for chunk c, the v quarter (every other c), the gate
        # chunk, then the (pipelined) attention iterations for chunk c.
        # The projection matmuls keep PE dense while attention's serial
        # ACT/DVE chains (exp -> sums -> recip -> rr) drain, which also
        # keeps the HAM clock-gate warm.
        p1 = p2 = None
        for c in range(NCH):
            w = proj_load_w(wq_d, c)
            proj_chunk_rope(w, qT, c)
            w = proj_load_w(wk_d, c)
            proj_chunk_rope(w, kT, c)
            if c % 4 == 0:
                nv = c // 4
                wvb = wvpool.tile([128, NCH, 512], F16, tag="wv")
                for kc in range(NCH):
                    nc.sync.dma_start(
                        wvb[:, kc, :],
                        wv_d[kc * 128:(kc + 1) * 128,
                             nv * 512:(nv + 1) * 512])
                for rc in range(NCH):
                    ps = big_ps.tile([128, 512], F32, tag="big")
                    for kc in range(NCH):
                        nc.tensor.matmul(
                            ps[:],
                            xt[:, kc, rc * 128:(rc + 1) * 128],
                            wvb[:, kc, :],
                            start=(kc == 0),
                            stop=(kc == NCH - 1),
                        )
                    vh = v[:, rc, :].rearrange("p (h t) -> p h t", t=65)
                    nc.vector.tensor_copy(
                        vh[:, 8 * nv:8 * nv + 8, 0:64],
                        ps[:].rearrange("p (h d) -> p h d", d=64))
            w = proj_load_w(wg_d, c)
            proj_chunk_gate(w, c)
            for b in range(NB):
                cur = attn_front(b, c)
                if p1 is not None:
                    m = attn_mid(p1)
                    if p2 is not None:
                        attn_tail(p2)
                    p2 = m
                p1 = cur
        m = attn_mid(p1)
        attn_tail(p2)
        attn_tail(m)

        # ---- output projection (transposed): outT[of, r] = Wo.T @ y
        for oc in range(NCH):
            w = proj_load_w(wo_d, oc)
            for nh in range(2):
                ps = big_ps.tile([128, 512], F32, tag="big")
                for yc in range(NCH):
                    nc.tensor.matmul(
                        ps[:],
                        w[:, yc, :],
                        sg[:, yc, nh * 512:(nh + 1) * 512],
                        start=(yc == 0),
                        stop=(yc == NCH - 1),
                    )
                osb = opool.tile([128, 512], F32, tag="o")
                nc.scalar.activation(osb[:], ps[:], AF.Copy)
                nc.sync.dma_start(
                    outT_d[oc * 128:(oc + 1) * 128,
                           nh * 512:(nh + 1) * 512], osb[:])


def _build_nc():
    import concourse.bacc as bacc
    import concourse.mybir as mybir
    import concourse.tile as tile

    F32 = mybir.dt.float32
    F16 = mybir.dt.float16
    nc = bacc.Bacc("TRN2", target_bir_lowering=False, debug=False)
    names_in = ["xT", "wq", "wk", "wv", "wg", "wo", "cos2", "sin2"]
    shapes_in = [[E, R], [E, E], [E, E], [E, E], [E, E], [E, E],
                 [128, R], [128, R]]
    dts_in = [F16, F16, F16, F16, F16, F16, F16, F16]
    ins = [
        nc.dram_tensor(n, s, dt, kind="ExternalInput").ap()
        for n, s, dt in zip(names_in, shapes_in, dts_in)
    ]
    outT = nc.dram_tensor("outT", [E, R], F32, kind="ExternalOutput").ap()
    with tile.TileContext(nc) as tc:
        emit(tc, [outT], ins)
    nc.compile()
    return nc


_NC_CACHE = {}


def host_prep(x, Wq, Wk, Wv, Wg, Wo, cos, sin):
    """Build the 8 per-core input maps."""
    x_flat = np.ascontiguousarray(x.reshape(B * S, E), dtype=np.float32)
    Wq = np.ascontiguousarray(Wq, dtype=np.float16)
    Wk = np.ascontiguousarray(Wk, dtype=np.float16)
    Wv = np.ascontiguousarray(Wv, dtype=np.float16)
    Wg = np.ascontiguousarray(Wg, dtype=np.float16)
    Wo = np.ascontiguousarray(Wo, dtype=np.float16)
    cos = np.asarray(cos, dtype=np.float32)
    sin = np.asarray(sin, dtype=np.float32)
    sign = np.where(np.arange(D) < D // 2, -1.0, 1.0).astype(np.float32)

    in_maps = []
    for cix in range(NCORES):
        rows = slice(cix * R, (cix + 1) * R)
        xT = np.ascontiguousarray(x_flat[rows].T.astype(np.float16))
        seq = (cix * R + np.arange(R)) % S
        cS = cos[seq]            # [R, D]
        sS = sin[seq] * sign     # [R, D] signed
        c2 = np.ascontiguousarray(np.tile(cS.T, (2, 1)).astype(np.float16))
        s2 = np.ascontiguousarray(np.tile(sS.T, (2, 1)).astype(np.float16))
        in_maps.append({
            "xT": xT, "wq": Wq, "wk": Wk, "wv": Wv, "wg": Wg, "wo": Wo,
            "cos2": c2, "sin2": s2,
        })
    return in_maps


def kernel_traced(x, Wq, Wk, Wv, Wg, Wo, cos, sin, block_size, trace=False,
                  **run_kwargs):
    assert int(block_size) == BLK
    from concourse import bass_utils

    if "nc" not in _NC_CACHE:
        _NC_CACHE["nc"] = _build_nc()
    nc = _NC_CACHE["nc"]

    in_maps = host_prep(x, Wq, Wk, Wv, Wg, Wo, cos, sin)
    res = bass_utils.run_bass_kernel_spmd(
        nc, in_maps, core_ids=list(range(NCORES)), trace=trace, **run_kwargs)
    out_flat = np.empty((B * S, E), dtype=np.float32)
    for cix in range(NCORES):
        out_flat[cix * R:(cix + 1) * R] = res.results[cix]["outT"].T
    return out_flat.reshape(B, S, E), res


def kernel(x, Wq, Wk, Wv, Wg, Wo, cos, sin, block_size):
    return kernel_traced(x, Wq, Wk, Wv, Wg, Wo, cos, sin, block_size)[0]


# revision 18
# speedup vs baseline: 1.0041x; 1.0041x over previous
"""Trainium2 Bass kernel for nn_BlockAttention (block-local attention with RoPE + gate).

Sharding: sequence-parallel over 8 cores. Flattened [B*S=8192, E] rows split into
8 contiguous shards of 1024 rows (4 blocks of 256; blocks never cross cores or
batch boundaries since 4096/256=16 blocks per batch, 4 per core).

Per-core layout strategy (features-on-partitions, "transposed" activations):
  - host pre-transposes the x shard to xT [E, R] so no on-chip transposes needed
  - qT/kT/gateT [E, R] = W.T @ x via matmul(lhsT=W_chunk, rhs=xT_chunk)
  - v [R, E] natural via matmul(lhsT=xT_chunk, rhs=Wv_chunk)
  - all matmul operands fp16/bf16: fp32r streams at half rate on HW and
    serializes an internal 4-byte weight load; fp16/bf16 get FWL + the
    separate LDWEIGHTS overlaps under the PE reorder window
  - RoPE applied on transposed q/k with host-prepared fp16 cos/sin tables
    (replicated per head-pair, rotate-sign folded into sin table); all DVE
    ops 16-bit for the 2x DVE rate
  - block-local attention per (block, head) with transposed scores S_T[k,q]:
    exp on ScalarE -> bf16 (bf16 range covers e^15; no max subtraction
    needed), AV via matmul(lhsT=v_block, rhs=expS_T) in bf16 with a
    trailing ones column per head so row 64 of the AV psum is the softmax
    row-sum (free), normalize via one K=1 ones-outer-product replicate
    matmul (bf16) + DVE muls
  - gate: tanh on ScalarE (same ACT table set as exp -- sigmoid would
    force a ~2.7us table reload per switch), sigmoid(g) = 0.5 + 0.5*tanh(g/2)
    finished by one fused DVE tensor_scalar; multiply fused into the
    attention tail
  - out projection back through Wo in transposed layout; host un-transposes
"""
import sys

sys.path.insert(0, "/opt/trn_rl_repo")
import numpy as np

B, S, E = 2, 4096, 1024
H, D = 16, 64
BLK = 256
NCORES = 8
R = (B * S) // NCORES   # 1024 rows per core
NB = R // BLK           # 4 blocks per core
NCH = E // 128          # 8 feature chunks of 128
SCALE = 1.0 / np.sqrt(D)


def emit(tc, outs, ins):
    """Emit the per-core program. ins/outs are DRAM APs:
    ins  = [xT, wq, wk, wv, wg, wo, cos2, sin2]
    outs = [outT]
    """
    from contextlib import ExitStack
    import concourse.mybir as mybir

    F32 = mybir.dt.float32
    F16 = mybir.dt.float16
    BF16 = mybir.dt.bfloat16
    AF = mybir.ActivationFunctionType
    ALU = mybir.AluOpType

    nc = tc.nc
    xT_d, wq_d, wk_d, wv_d, wg_d, wo_d, c2_d, s2_d = ins
    (outT_d,) = outs

    with ExitStack() as ctx:
        ep = ctx.enter_context
        consts = ep(tc.tile_pool(name="consts", bufs=1))
        big = ep(tc.tile_pool(name="big", bufs=1))
        wpool = ep(tc.tile_pool(name="wpool", bufs=3))
        wvpool = ep(tc.tile_pool(name="wvpool", bufs=2))
        ropet = ep(tc.tile_pool(name="ropet", bufs=2))
        rawp = ep(tc.tile_pool(name="rawp", bufs=2))
        espool = ep(tc.tile_pool(name="espool", bufs=4))
        smalls = ep(tc.tile_pool(name="smalls", bufs=2))
        rrsbp = ep(tc.tile_pool(name="rrsbp", bufs=2))
        ytp = ep(tc.tile_pool(name="ytp", bufs=2))
        opool = ep(tc.tile_pool(name="opool", bufs=2))
        # PSUM: 8 banks total, everything double-buffered.
        big_ps = ep(tc.tile_pool(name="big_ps", bufs=3, space="PSUM"))
        s_ps_p = ep(tc.tile_pool(name="s_ps_p", bufs=2, space="PSUM"))
        av_ps_p = ep(tc.tile_pool(name="av_ps_p", bufs=2, space="PSUM"))
        rr_ps_p = ep(tc.tile_pool(name="rr_ps_p", bufs=1, space="PSUM"))

        # ---- constants / inputs resident in SBUF
        # DMA issue order = queue order: the first projection group needs
        # the wq chunk-0 weights and rows 0:512 of every xt kc chunk, so
        # those go first; second xt halves and the rope tables follow.
        def proj_load_w(w_d, mc):
            w = wpool.tile([128, NCH, 128], F16, tag="w")
            src = w_d.rearrange("(kc p) m -> p kc m", p=128)
            nc.sync.dma_start(w[:], src[:, :, mc * 128:(mc + 1) * 128])
            return w

        w_first = proj_load_w(wq_d, 0)
        xt = big.tile([128, NCH, R], F16)
        for kc in range(NCH):
            nc.sync.dma_start(xt[:, kc, 0:512], xT_d[kc * 128:(kc + 1) * 128, 0:512])
        for kc in range(NCH):
            nc.sync.dma_start(xt[:, kc, 512:R], xT_d[kc * 128:(kc + 1) * 128, 512:R])
        c2 = consts.tile([128, R], F16)
        nc.sync.dma_start(c2[:], c2_d[:])
        s2 = consts.tile([128, R], F16)
        nc.sync.dma_start(s2[:], s2_d[:])
        onesrowf = consts.tile([1, 64], F32)
        nc.vector.memset(onesrowf[:], 1.0)
        onesrow = consts.tile([1, 64], BF16)
        nc.vector.tensor_copy(onesrow[:], onesrowf[:])

        qT = big.tile([128, NCH, R], F16)
        kT = big.tile([128, NCH, R], F16)
        # v holds 16 heads x (64 dims + a ones column) per row-chunk: the
        # ones column makes each AV matmul also emit the softmax row-sums
        # (output row 64) for free.
        v = big.tile([128, NCH, H * 65], BF16)
        ones16f = consts.tile([128, 16], F32)
        nc.vector.memset(ones16f[:], 1.0)
        # sg doubles as y: the attention tail multiplies the gate in-place
        # (av*rr*sg), and the out projection consumes it. fp16: it feeds
        # the fp16 out-projection matmul.
        sg = big.tile([128, NCH, R], F16)

        # ---- one projection output chunk: 8-matmul psum group + drain
        def mm_group(w, nh):
            ps = big_ps.tile([128, 512], F32, tag="big")
            for kc in range(NCH):
                nc.tensor.matmul(
                    ps[:],
                    w[:, kc, :],
                    xt[:, kc, nh * 512:(nh + 1) * 512],
                    start=(kc == 0),
                    stop=(kc == NCH - 1),
                )
            return ps

        def proj_chunk_gate(w, mc):
            # raw-gate tanh(g/2); finished to sigmoid by one fused DVE op
            for nh in range(2):
                ps = mm_group(w, nh)
                dstsl = sg[:, mc, nh * 512:(nh + 1) * 512]
                nc.scalar.activation(dstsl, ps[:], AF.Tanh, scale=0.5)
            # sigmoid(g) = 0.5*tanh(g/2) + 0.5, fp16 in-place at 2x DVE rate
            nc.vector.tensor_scalar(
                sg[:, mc, :], sg[:, mc, :], 0.5, 0.5, ALU.mult, ALU.add)

        def proj_chunk_rope(w, dst, mc):
            # RoPE: the rotate (partition swap d <-> d+-32 within each head)
            # rides on the otherwise-idle DMA engines as 4 partition-shifted
            # SBUF->SBUF copies of the raw projection; sign lives in sin2.
            ps0 = mm_group(w, 0)
            ps1 = mm_group(w, 1)
            raw = rawp.tile([128, R], F16, tag="raw")
            nc.scalar.activation(raw[:, 0:512], ps0[:], AF.Copy)
            nc.scalar.activation(raw[:, 512:1024], ps1[:], AF.Copy)
            t = ropet.tile([128, R], F16, tag="t")
            for h2 in (0, 64):
                nc.sync.dma_start(t[h2:h2 + 32, :], raw[h2 + 32:h2 + 64, :])
                nc.sync.dma_start(t[h2 + 32:h2 + 64, :], raw[h2:h2 + 32, :])
            dsl = dst[:, mc, :]
            nc.vector.tensor_mul(dsl, raw[:], c2[:])
            nc.vector.tensor_mul(t[:], t[:], s2[:])
            nc.vector.tensor_add(dsl, dsl, t[:])

        def attn_front(b, c):
            est = []
            for hi in range(2):
                pb = 64 * hi
                sps = s_ps_p.tile([128, 512], F32, tag="s")
                for kph in range(2):
                    nc.tensor.matmul(
                        sps[:, kph * 256:(kph + 1) * 256],
                        kT[pb:pb + 64, c,
                           b * 256 + kph * 128:b * 256 + (kph + 1) * 128],
                        qT[pb:pb + 64, c, b * 256:(b + 1) * 256],
                        start=True, stop=True,
                    )
                es = espool.tile([128, 512], BF16, tag="es")
                nc.scalar.activation(es[:], sps[:], AF.Exp,
                                     scale=float(SCALE))
                est.append(es)
            return (b, c, est)

        def attn_mid(st):
            b, c, est = st
            # both heads' AV share one psum bank: hi=0 in cols 0:256,
            # hi=1 in cols 256:512; row 0 = softmax row-sums (ones-led V)
            av = av_ps_p.tile([65, 512], F32, tag="av")
            for hi in range(2):
                es = est[hi]
                h = 2 * c + hi
                for kph in range(2):
                    nc.tensor.matmul(
                        av[:, hi * 256:(hi + 1) * 256],
                        v[:, 2 * b + kph, h * 65:(h + 1) * 65],
                        es[:, kph * 256:(kph + 1) * 256],
                        start=(kph == 0), stop=(kph == 1),
                    )
            # PSUM reads at base partition 64 misread on HW (both the DVE
            # custom reciprocal and ScalarE copies) -- stage the sums row
            # through SBUF via a plain DVE copy, which is safe.
            sumst = smalls.tile([1, 512], F32, tag="sumst")
            nc.vector.tensor_copy(sumst[:], av[64:65, :])
            recipf = smalls.tile([1, 512], F32, tag="recipf")
            nc.vector.reciprocal_approx_fast(recipf[:], sumst[:])
            recip = smalls.tile([1, 512], BF16, tag="recip")
            nc.scalar.activation(recip[:], recipf[:], AF.Copy)
            return (b, c, av, recip)

        def attn_tail(st):
            b, c, av, recip = st
            # replicate recip over the 64 head dims via a K=1
            # ones-outer-product matmul, drained by ScalarE
            rr = rr_ps_p.tile([64, 512], F32, tag="rr")
            nc.tensor.matmul(rr[:], onesrow[:], recip[0:1, :],
                             start=True, stop=True)
            rrsb = rrsbp.tile([128, 256], F32, tag="rrsb")
            nc.scalar.activation(rrsb[0:64, :], rr[0:64, 0:256], AF.Copy)
            nc.scalar.activation(rrsb[64:128, :], rr[0:64, 256:512], AF.Copy)
            # y = av * rr * sg, written in-place over sg
            yt = ytp.tile([128, 256], F16, tag="yt")
            for hi in range(2):
                ysl_p = slice(64 * hi, 64 * hi + 64)
                nc.vector.tensor_mul(yt[ysl_p, :],
                                     av[0:64, hi * 256:(hi + 1) * 256],
                                     rrsb[ysl_p, :])
                ysl = sg[ysl_p, c, b * 256:(b + 1) * 256]
                nc.vector.tensor_mul(ysl, ysl, yt[ysl_p, :])

        for rc in range(NCH):
            vh = v[:, rc, :].rearrange("p (h t) -> p h t", t=65)
            nc.scalar.activation(vh[:, :, 64], ones16f[:], AF.Copy)

        # ---- fused main loop: per chunk c, emit the q/qrot/k/krot
        # projections for chunk c, the v quarter (every other c), the gate
        # chunk, then the (pipelined) attention iterations for chunk c.
        # The projection matmuls keep PE dense while attention's serial
        # ACT/DVE chains (exp -> sums -> recip -> rr) drain, which also
        # keeps the HAM clock-gate warm.
        p1 = p2 = None
        for c in range(NCH):
            w = w_first if c == 0 else proj_load_w(wq_d, c)
            proj_chunk_rope(w, qT, c)
            w = proj_load_w(wk_d, c)
            proj_chunk_rope(w, kT, c)
            if c % 4 == 0:
                nv = c // 4
                wvb = wvpool.tile([128, NCH, 512], F16, tag="wv")
                for kc in range(NCH):
                    nc.sync.dma_start(
                        wvb[:, kc, :],
                        wv_d[kc * 128:(kc + 1) * 128,
                             nv * 512:(nv + 1) * 512])
                for rc in range(NCH):
                    ps = big_ps.tile([128, 512], F32, tag="big")
                    for kc in range(NCH):
                        nc.tensor.matmul(
                            ps[:],
                            xt[:, kc, rc * 128:(rc + 1) * 128],
                            wvb[:, kc, :],
                            start=(kc == 0),
                            stop=(kc == NCH - 1),
                        )
                    vh = v[:, rc, :].rearrange("p (h t) -> p h t", t=65)
                    nc.vector.tensor_copy(
                        vh[:, 8 * nv:8 * nv + 8, 0:64],
                        ps[:].rearrange("p (h d) -> p h d", d=64))
            w = proj_load_w(wg_d, c)
            proj_chunk_gate(w, c)
            for b in range(NB):
                cur = attn_front(b, c)
                if p1 is not None:
                    m = attn_mid(p1)
                    if p2 is not None:
                        attn_tail(p2)
                    p2 = m
                p1 = cur
        m = attn_mid(p1)
        attn_tail(p2)
        attn_tail(m)

        # ---- output projection (transposed): outT[of, r] = Wo.T @ y
        for oc in range(NCH):
            w = proj_load_w(wo_d, oc)
            for nh in range(2):
                ps = big_ps.tile([128, 512], F32, tag="big")
                for yc in range(NCH):
                    nc.tensor.matmul(
                        ps[:],
                        w[:, yc, :],
                        sg[:, yc, nh * 512:(nh + 1) * 512],
                        start=(yc == 0),
                        stop=(yc == NCH - 1),
                    )
                osb = opool.tile([128, 512], F32, tag="o")
                nc.scalar.activation(osb[:], ps[:], AF.Copy)
                nc.sync.dma_start(
                    outT_d[oc * 128:(oc + 1) * 128,
                           nh * 512:(nh + 1) * 512], osb[:])


def _build_nc():
    import concourse.bacc as bacc
    import concourse.mybir as mybir
    import concourse.tile as tile

    F32 = mybir.dt.float32
    F16 = mybir.dt.float16
    nc = bacc.Bacc("TRN2", target_bir_lowering=False, debug=False)
    names_in = ["xT", "wq", "wk", "wv", "wg", "wo", "cos2", "sin2"]
    shapes_in = [[E, R], [E, E], [E, E], [E, E], [E, E], [E, E],
                 [128, R], [128, R]]
    dts_in = [F16, F16, F16, F16, F16, F16, F16, F16]
    ins = [
        nc.dram_tensor(n, s, dt, kind="ExternalInput").ap()
        for n, s, dt in zip(names_in, shapes_in, dts_in)
    ]
    outT = nc.dram_tensor("outT", [E, R], F32, kind="ExternalOutput").ap()
    with tile.TileContext(nc) as tc:
        emit(tc, [outT], ins)
    nc.compile()
    return nc


_NC_CACHE = {}


def host_prep(x, Wq, Wk, Wv, Wg, Wo, cos, sin):
    """Build the 8 per-core input maps."""
    x_flat = np.ascontiguousarray(x.reshape(B * S, E), dtype=np.float32)
    Wq = np.ascontiguousarray(Wq, dtype=np.float16)
    Wk = np.ascontiguousarray(Wk, dtype=np.float16)
    Wv = np.ascontiguousarray(Wv, dtype=np.float16)
    Wg = np.ascontiguousarray(Wg, dtype=np.float16)
    Wo = np.ascontiguousarray(Wo, dtype=np.float16)
    cos = np.asarray(cos, dtype=np.float32)
    sin = np.asarray(sin, dtype=np.float32)
    sign = np.where(np.arange(D) < D // 2, -1.0, 1.0).astype(np.float32)

    in_maps = []
    for cix in range(NCORES):
        rows = slice(cix * R, (cix + 1) * R)
        xT = np.ascontiguousarray(x_flat[rows].T.astype(np.float16))
        seq = (cix * R + np.arange(R)) % S
        cS = cos[seq]            # [R, D]
        sS = sin[seq] * sign     # [R, D] signed
        c2 = np.ascontiguousarray(np.tile(cS.T, (2, 1)).astype(np.float16))
        s2 = np.ascontiguousarray(np.tile(sS.T, (2, 1)).astype(np.float16))
        in_maps.append({
            "xT": xT, "wq": Wq, "wk": Wk, "wv": Wv, "wg": Wg, "wo": Wo,
            "cos2": c2, "sin2": s2,
        })
    return in_maps


def kernel_traced(x, Wq, Wk, Wv, Wg, Wo, cos, sin, block_size, trace=False,
                  **run_kwargs):
    assert int(block_size) == BLK
    from concourse import bass_utils

    if "nc" not in _NC_CACHE:
        _NC_CACHE["nc"] = _build_nc()
    nc = _NC_CACHE["nc"]

    in_maps = host_prep(x, Wq, Wk, Wv, Wg, Wo, cos, sin)
    res = bass_utils.run_bass_kernel_spmd(
        nc, in_maps, core_ids=list(range(NCORES)), trace=trace, **run_kwargs)
    out_flat = np.empty((B * S, E), dtype=np.float32)
    for cix in range(NCORES):
        out_flat[cix * R:(cix + 1) * R] = res.results[cix]["outT"].T
    return out_flat.reshape(B, S, E), res


def kernel(x, Wq, Wk, Wv, Wg, Wo, cos, sin, block_size):
    return kernel_traced(x, Wq, Wk, Wv, Wg, Wo, cos, sin, block_size)[0]


# revision 20
# speedup vs baseline: 1.0359x; 1.0317x over previous
"""Trainium2 Bass kernel for nn_BlockAttention (block-local attention with RoPE + gate).

Sharding: sequence-parallel over 8 cores. Flattened [B*S=8192, E] rows split into
8 contiguous shards of 1024 rows (4 blocks of 256; blocks never cross cores or
batch boundaries since 4096/256=16 blocks per batch, 4 per core).

Per-core layout strategy (features-on-partitions, "transposed" activations):
  - host pre-transposes the x shard to xT [E, R] so no on-chip transposes needed
  - qT/kT/gateT [E, R] = W.T @ x via matmul(lhsT=W_chunk, rhs=xT_chunk)
  - v [R, E] natural via matmul(lhsT=xT_chunk, rhs=Wv_chunk)
  - all matmul operands fp16/bf16: fp32r streams at half rate on HW and
    serializes an internal 4-byte weight load; fp16/bf16 get FWL + the
    separate LDWEIGHTS overlaps under the PE reorder window
  - RoPE applied on transposed q/k with host-prepared fp16 cos/sin tables
    (replicated per head-pair, rotate-sign folded into sin table); all DVE
    ops 16-bit for the 2x DVE rate
  - block-local attention per (block, head) with transposed scores S_T[k,q]:
    exp on ScalarE -> bf16 (bf16 range covers e^15; no max subtraction
    needed), AV via matmul(lhsT=v_block, rhs=expS_T) in bf16 with a
    trailing ones column per head so row 64 of the AV psum is the softmax
    row-sum (free), normalize via one K=1 ones-outer-product replicate
    matmul (bf16) + DVE muls
  - gate: tanh on ScalarE (same ACT table set as exp -- sigmoid would
    force a ~2.7us table reload per switch), sigmoid(g) = 0.5 + 0.5*tanh(g/2)
    finished by one fused DVE tensor_scalar; multiply fused into the
    attention tail
  - out projection back through Wo in transposed layout; host un-transposes
"""
import sys

sys.path.insert(0, "/opt/trn_rl_repo")
import numpy as np

B, S, E = 2, 4096, 1024
H, D = 16, 64
BLK = 256
NCORES = 8
R = (B * S) // NCORES   # 1024 rows per core
NB = R // BLK           # 4 blocks per core
NCH = E // 128          # 8 feature chunks of 128
SCALE = 1.0 / np.sqrt(D)


def emit(tc, outs, ins):
    """Emit the per-core program. ins/outs are DRAM APs:
    ins  = [xT, wq, wk, wv, wg, wo, cos2, sin2]
    outs = [outT]
    """
    from contextlib import ExitStack
    import concourse.mybir as mybir

    F32 = mybir.dt.float32
    F16 = mybir.dt.float16
    BF16 = mybir.dt.bfloat16
    AF = mybir.ActivationFunctionType
    ALU = mybir.AluOpType

    nc = tc.nc
    xT_d, wq_d, wk_d, wv_d, wg_d, wo_d, c2_d, s2_d = ins
    (outT_d,) = outs

    with ExitStack() as ctx:
        ep = ctx.enter_context
        consts = ep(tc.tile_pool(name="consts", bufs=1))
        big = ep(tc.tile_pool(name="big", bufs=1))
        wpool = ep(tc.tile_pool(name="wpool", bufs=3))
        wvpool = ep(tc.tile_pool(name="wvpool", bufs=2))
        ropet = ep(tc.tile_pool(name="ropet", bufs=2))
        rawp = ep(tc.tile_pool(name="rawp", bufs=2))
        espool = ep(tc.tile_pool(name="espool", bufs=4))
        smalls = ep(tc.tile_pool(name="smalls", bufs=2))
        rrsbp = ep(tc.tile_pool(name="rrsbp", bufs=2))
        ytp = ep(tc.tile_pool(name="ytp", bufs=2))
        opool = ep(tc.tile_pool(name="opool", bufs=2))
        # PSUM: 8 banks total, everything double-buffered.
        big_ps = ep(tc.tile_pool(name="big_ps", bufs=3, space="PSUM"))
        s_ps_p = ep(tc.tile_pool(name="s_ps_p", bufs=2, space="PSUM"))
        av_ps_p = ep(tc.tile_pool(name="av_ps_p", bufs=3, space="PSUM"))

        # ---- constants / inputs resident in SBUF
        # DMA issue order = queue order: the first projection group needs
        # the wq chunk-0 weights and rows 0:512 of every xt kc chunk, so
        # those go first; second xt halves and the rope tables follow.
        def proj_load_w(w_d, mc):
            w = wpool.tile([128, NCH, 128], F16, tag="w")
            src = w_d.rearrange("(kc p) m -> p kc m", p=128)
            nc.sync.dma_start(w[:], src[:, :, mc * 128:(mc + 1) * 128])
            return w

        w_first = proj_load_w(wq_d, 0)
        xt = big.tile([128, NCH, R], F16)
        for kc in range(NCH):
            nc.sync.dma_start(xt[:, kc, 0:512], xT_d[kc * 128:(kc + 1) * 128, 0:512])
        for kc in range(NCH):
            nc.sync.dma_start(xt[:, kc, 512:R], xT_d[kc * 128:(kc + 1) * 128, 512:R])
        c2 = consts.tile([128, R], F16)
        nc.sync.dma_start(c2[:], c2_d[:])
        s2 = consts.tile([128, R], F16)
        nc.sync.dma_start(s2[:], s2_d[:])

        qT = big.tile([128, NCH, R], F16)
        kT = big.tile([128, NCH, R], F16)
        # v holds 16 heads x (64 dims + a ones column) per row-chunk: the
        # ones column makes each AV matmul also emit the softmax row-sums
        # (output row 64) for free.
        v = big.tile([128, NCH, H * 65], BF16)
        ones16f = consts.tile([128, 16], F32)
        nc.vector.memset(ones16f[:], 1.0)
        # sg doubles as y: the attention tail multiplies the gate in-place
        # (av*rr*sg), and the out projection consumes it. fp16: it feeds
        # the fp16 out-projection matmul.
        sg = big.tile([128, NCH, R], F16)

        # ---- one projection output chunk: 8-matmul psum group + drain
        def mm_group(w, nh):
            ps = big_ps.tile([128, 512], F32, tag="big")
            for kc in range(NCH):
                nc.tensor.matmul(
                    ps[:],
                    w[:, kc, :],
                    xt[:, kc, nh * 512:(nh + 1) * 512],
                    start=(kc == 0),
                    stop=(kc == NCH - 1),
                )
            return ps

        def proj_chunk_gate(w, mc):
            # raw-gate tanh(g/2); finished to sigmoid by one fused DVE op
            for nh in range(2):
                ps = mm_group(w, nh)
                dstsl = sg[:, mc, nh * 512:(nh + 1) * 512]
                nc.scalar.activation(dstsl, ps[:], AF.Tanh, scale=0.5)
            # sigmoid(g) = 0.5*tanh(g/2) + 0.5, fp16 in-place at 2x DVE rate
            nc.vector.tensor_scalar(
                sg[:, mc, :], sg[:, mc, :], 0.5, 0.5, ALU.mult, ALU.add)

        def proj_chunk_rope(w, dst, mc):
            # RoPE: the rotate (partition swap d <-> d+-32 within each head)
            # rides on the otherwise-idle DMA engines as 4 partition-shifted
            # SBUF->SBUF copies of the raw projection; sign lives in sin2.
            ps0 = mm_group(w, 0)
            ps1 = mm_group(w, 1)
            raw = rawp.tile([128, R], F16, tag="raw")
            nc.scalar.activation(raw[:, 0:512], ps0[:], AF.Copy)
            nc.scalar.activation(raw[:, 512:1024], ps1[:], AF.Copy)
            t = ropet.tile([128, R], F16, tag="t")
            for h2 in (0, 64):
                nc.sync.dma_start(t[h2:h2 + 32, :], raw[h2 + 32:h2 + 64, :])
                nc.sync.dma_start(t[h2 + 32:h2 + 64, :], raw[h2:h2 + 32, :])
            dsl = dst[:, mc, :]
            nc.vector.tensor_mul(dsl, raw[:], c2[:])
            nc.vector.tensor_mul(t[:], t[:], s2[:])
            nc.vector.tensor_add(dsl, dsl, t[:])

        def attn_front(b, c):
            est = []
            for hi in range(2):
                pb = 64 * hi
                sps = s_ps_p.tile([128, 512], F32, tag="s")
                for kph in range(2):
                    nc.tensor.matmul(
                        sps[:, kph * 256:(kph + 1) * 256],
                        kT[pb:pb + 64, c,
                           b * 256 + kph * 128:b * 256 + (kph + 1) * 128],
                        qT[pb:pb + 64, c, b * 256:(b + 1) * 256],
                        start=True, stop=True,
                    )
                es = espool.tile([128, 512], BF16, tag="es")
                nc.scalar.activation(es[:], sps[:], AF.Exp,
                                     scale=float(SCALE))
                est.append(es)
            return (b, c, est)

        def attn_mid(st):
            b, c, est = st
            # both heads' AV share one psum bank: hi=0 in cols 0:256,
            # hi=1 in cols 256:512; row 0 = softmax row-sums (ones-led V)
            av = av_ps_p.tile([65, 512], F32, tag="av")
            for hi in range(2):
                es = est[hi]
                h = 2 * c + hi
                for kph in range(2):
                    nc.tensor.matmul(
                        av[:, hi * 256:(hi + 1) * 256],
                        v[:, 2 * b + kph, h * 65:(h + 1) * 65],
                        es[:, kph * 256:(kph + 1) * 256],
                        start=(kph == 0), stop=(kph == 1),
                    )
            # PSUM reads at base partition 64 misread on HW (both the DVE
            # custom reciprocal and ScalarE copies) -- stage the sums row
            # through SBUF via a plain DVE copy, which is safe.
            sumst = smalls.tile([1, 512], F32, tag="sumst")
            nc.vector.tensor_copy(sumst[:], av[64:65, :])
            recipf = smalls.tile([1, 512], F32, tag="recipf")
            nc.vector.reciprocal_approx_fast(recipf[:], sumst[:])
            # replicate recip over the 64 head dims on the idle Pool
            # engine. The Q7 broadcast maps DSP core j to partitions
            # [16j, 16j+16) with an absolute-partition mask, so the
            # destination MUST start at partition 0 -- use two base-0
            # tiles (one per head) instead of one [128, 256] tile.
            rra = rrsbp.tile([64, 256], F32, tag="rra")
            rrb = rrsbp.tile([64, 256], F32, tag="rrb")
            nc.gpsimd.partition_broadcast(rra[:], recipf[0:1, 0:256])
            nc.gpsimd.partition_broadcast(rrb[:], recipf[0:1, 256:512])
            return (b, c, av, (rra, rrb))

        def attn_tail(st):
            b, c, av, rr2 = st
            # y = av * rr * sg, written in-place over sg
            yt = ytp.tile([128, 256], F16, tag="yt")
            for hi in range(2):
                ysl_p = slice(64 * hi, 64 * hi + 64)
                nc.vector.tensor_mul(yt[ysl_p, :],
                                     av[0:64, hi * 256:(hi + 1) * 256],
                                     rr2[hi][0:64, :])
                ysl = sg[ysl_p, c, b * 256:(b + 1) * 256]
                nc.vector.tensor_mul(ysl, ysl, yt[ysl_p, :])

        for rc in range(NCH):
            vh = v[:, rc, :].rearrange("p (h t) -> p h t", t=65)
            nc.scalar.activation(vh[:, :, 64], ones16f[:], AF.Copy)

        # ---- fused main loop: per chunk c, emit the q/qrot/k/krot
        # projections for chunk c, the v quarter (every other c), the gate
        # chunk, then the (pipelined) attention iterations for chunk c.
        # The projection matmuls keep PE dense while attention's serial
        # ACT/DVE chains (exp -> sums -> recip -> rr) drain, which also
        # keeps the HAM clock-gate warm.
        p1 = p2 = None
        for c in range(NCH):
            w = w_first if c == 0 else proj_load_w(wq_d, c)
            proj_chunk_rope(w, qT, c)
            w = proj_load_w(wk_d, c)
            proj_chunk_rope(w, kT, c)
            if c % 4 == 0:
                nv = c // 4
                wvb = wvpool.tile([128, NCH, 512], F16, tag="wv")
                for kc in range(NCH):
                    nc.sync.dma_start(
                        wvb[:, kc, :],
                        wv_d[kc * 128:(kc + 1) * 128,
                             nv * 512:(nv + 1) * 512])
                for rc in range(NCH):
                    ps = big_ps.tile([128, 512], F32, tag="big")
                    for kc in range(NCH):
                        nc.tensor.matmul(
                            ps[:],
                            xt[:, kc, rc * 128:(rc + 1) * 128],
                            wvb[:, kc, :],
                            start=(kc == 0),
                            stop=(kc == NCH - 1),
                        )
                    vh = v[:, rc, :].rearrange("p (h t) -> p h t", t=65)
                    nc.vector.tensor_copy(
                        vh[:, 8 * nv:8 * nv + 8, 0:64],
                        ps[:].rearrange("p (h d) -> p h d", d=64))
            w = proj_load_w(wg_d, c)
            proj_chunk_gate(w, c)
            for b in range(NB):
                cur = attn_front(b, c)
                if p1 is not None:
                    m = attn_mid(p1)
                    if p2 is not None:
                        attn_tail(p2)
                    p2 = m
                p1 = cur
        m = attn_mid(p1)
        attn_tail(p2)
        attn_tail(m)

        # ---- output projection (transposed): outT[of, r] = Wo.T @ y
        for oc in range(NCH):
            w = proj_load_w(wo_d, oc)
            for nh in range(2):
                ps = big_ps.tile([128, 512], F32, tag="big")
                for yc in range(NCH):
                    nc.tensor.matmul(
                        ps[:],
                        w[:, yc, :],
                        sg[:, yc, nh * 512:(nh + 1) * 512],
                        start=(yc == 0),
                        stop=(yc == NCH - 1),
                    )
                osb = opool.tile([128, 512], F32, tag="o")
                nc.scalar.activation(osb[:], ps[:], AF.Copy)
                nc.sync.dma_start(
                    outT_d[oc * 128:(oc + 1) * 128,
                           nh * 512:(nh + 1) * 512], osb[:])


def _build_nc():
    import concourse.bacc as bacc
    import concourse.mybir as mybir
    import concourse.tile as tile

    F32 = mybir.dt.float32
    F16 = mybir.dt.float16
    nc = bacc.Bacc("TRN2", target_bir_lowering=False, debug=False)
    names_in = ["xT", "wq", "wk", "wv", "wg", "wo", "cos2", "sin2"]
    shapes_in = [[E, R], [E, E], [E, E], [E, E], [E, E], [E, E],
                 [128, R], [128, R]]
    dts_in = [F16, F16, F16, F16, F16, F16, F16, F16]
    ins = [
        nc.dram_tensor(n, s, dt, kind="ExternalInput").ap()
        for n, s, dt in zip(names_in, shapes_in, dts_in)
    ]
    outT = nc.dram_tensor("outT", [E, R], F32, kind="ExternalOutput").ap()
    with tile.TileContext(nc) as tc:
        emit(tc, [outT], ins)
    nc.compile()
    return nc


_NC_CACHE = {}


def host_prep(x, Wq, Wk, Wv, Wg, Wo, cos, sin):
    """Build the 8 per-core input maps."""
    x_flat = np.ascontiguousarray(x.reshape(B * S, E), dtype=np.float32)
    Wq = np.ascontiguousarray(Wq, dtype=np.float16)
    Wk = np.ascontiguousarray(Wk, dtype=np.float16)
    Wv = np.ascontiguousarray(Wv, dtype=np.float16)
    Wg = np.ascontiguousarray(Wg, dtype=np.float16)
    Wo = np.ascontiguousarray(Wo, dtype=np.float16)
    cos = np.asarray(cos, dtype=np.float32)
    sin = np.asarray(sin, dtype=np.float32)
    sign = np.where(np.arange(D) < D // 2, -1.0, 1.0).astype(np.float32)

    in_maps = []
    for cix in range(NCORES):
        rows = slice(cix * R, (cix + 1) * R)
        xT = np.ascontiguousarray(x_flat[rows].T.astype(np.float16))
        seq = (cix * R + np.arange(R)) % S
        cS = cos[seq]            # [R, D]
        sS = sin[seq] * sign     # [R, D] signed
        c2 = np.ascontiguousarray(np.tile(cS.T, (2, 1)).astype(np.float16))
        s2 = np.ascontiguousarray(np.tile(sS.T, (2, 1)).astype(np.float16))
        in_maps.append({
            "xT": xT, "wq": Wq, "wk": Wk, "wv": Wv, "wg": Wg, "wo": Wo,
            "cos2": c2, "sin2": s2,
        })
    return in_maps


def kernel_traced(x, Wq, Wk, Wv, Wg, Wo, cos, sin, block_size, trace=False,
                  **run_kwargs):
    assert int(block_size) == BLK
    from concourse import bass_utils

    if "nc" not in _NC_CACHE:
        _NC_CACHE["nc"] = _build_nc()
    nc = _NC_CACHE["nc"]

    in_maps = host_prep(x, Wq, Wk, Wv, Wg, Wo, cos, sin)
    res = bass_utils.run_bass_kernel_spmd(
        nc, in_maps, core_ids=list(range(NCORES)), trace=trace, **run_kwargs)
    out_flat = np.empty((B * S, E), dtype=np.float32)
    for cix in range(NCORES):
        out_flat[cix * R:(cix + 1) * R] = res.results[cix]["outT"].T
    return out_flat.reshape(B, S, E), res


def kernel(x, Wq, Wk, Wv, Wg, Wo, cos, sin, block_size):
    return kernel_traced(x, Wq, Wk, Wv, Wg, Wo, cos, sin, block_size)[0]


# revision 22
# speedup vs baseline: 1.2113x; 1.1693x over previous
"""Trainium2 Bass kernel for nn_BlockAttention (block-local attention with RoPE + gate).

Sharding: sequence-parallel over 8 cores. Flattened [B*S=8192, E] rows split into
8 contiguous shards of 1024 rows (4 blocks of 256; blocks never cross cores or
batch boundaries since 4096/256=16 blocks per batch, 4 per core).

Per-core layout strategy (features-on-partitions, "transposed" activations):
  - host pre-transposes the x shard to xT [E, R] so no on-chip transposes needed
  - qT/kT/gateT [E, R] = W.T @ x via matmul(lhsT=W_chunk, rhs=xT_chunk)
  - v [R, E] natural via matmul(lhsT=xT_chunk, rhs=Wv_chunk)
  - all matmul operands fp16/bf16: fp32r streams at half rate on HW and
    serializes an internal 4-byte weight load; fp16/bf16 get FWL + the
    separate LDWEIGHTS overlaps under the PE reorder window
  - RoPE applied on transposed q/k with host-prepared fp16 cos/sin tables
    (replicated per head-pair, rotate-sign folded into sin table); all DVE
    ops 16-bit for the 2x DVE rate
  - block-local attention per (block, head) with transposed scores S_T[k,q]:
    exp on ScalarE -> bf16 (bf16 range covers e^15; no max subtraction
    needed), AV via matmul(lhsT=v_block, rhs=expS_T) in bf16 with a
    trailing ones column per head so row 64 of the AV psum is the softmax
    row-sum (free), normalize via one K=1 ones-outer-product replicate
    matmul (bf16) + DVE muls
  - gate: tanh on ScalarE (same ACT table set as exp -- sigmoid would
    force a ~2.7us table reload per switch), sigmoid(g) = 0.5 + 0.5*tanh(g/2)
    finished by one fused DVE tensor_scalar; multiply fused into the
    attention tail
  - out projection back through Wo in transposed layout; host un-transposes
"""
import sys

sys.path.insert(0, "/opt/trn_rl_repo")
import numpy as np

B, S, E = 2, 4096, 1024
H, D = 16, 64
BLK = 256
NCORES = 8
R = (B * S) // NCORES   # 1024 rows per core
NB = R // BLK           # 4 blocks per core
NCH = E // 128          # 8 feature chunks of 128
SCALE = 1.0 / np.sqrt(D)


def emit(tc, outs, ins):
    """Emit the per-core program. ins/outs are DRAM APs:
    ins  = [xT, wq, wk, wv, wg, wo, cos2, sin2]
    outs = [outT]
    """
    from contextlib import ExitStack
    import concourse.mybir as mybir

    F32 = mybir.dt.float32
    F16 = mybir.dt.float16
    BF16 = mybir.dt.bfloat16
    AF = mybir.ActivationFunctionType
    ALU = mybir.AluOpType

    nc = tc.nc
    xT_d, wq_d, wk_d, wv_d, wg_d, wo_d, c2_d, s2_d = ins
    (outT_d,) = outs

    with ExitStack() as ctx:
        ep = ctx.enter_context
        consts = ep(tc.tile_pool(name="consts", bufs=1))
        big = ep(tc.tile_pool(name="big", bufs=1))
        wpool = ep(tc.tile_pool(name="wpool", bufs=3))
        wvpool = ep(tc.tile_pool(name="wvpool", bufs=2))
        ropet = ep(tc.tile_pool(name="ropet", bufs=2))
        rawp = ep(tc.tile_pool(name="rawp", bufs=2))
        espool = ep(tc.tile_pool(name="espool", bufs=4))
        smalls = ep(tc.tile_pool(name="smalls", bufs=2))
        rrsbp = ep(tc.tile_pool(name="rrsbp", bufs=2))
        ytp = ep(tc.tile_pool(name="ytp", bufs=2))
        opool = ep(tc.tile_pool(name="opool", bufs=2))
        # PSUM: 8 banks total, everything double-buffered.
        big_ps = ep(tc.tile_pool(name="big_ps", bufs=3, space="PSUM"))
        s_ps_p = ep(tc.tile_pool(name="s_ps_p", bufs=2, space="PSUM"))
        av_ps_p = ep(tc.tile_pool(name="av_ps_p", bufs=3, space="PSUM"))

        # ---- constants / inputs resident in SBUF
        # DMA issue order = queue order: the first projection group needs
        # the wq chunk-0 weights and rows 0:512 of every xt kc chunk, so
        # those go first; second xt halves and the rope tables follow.
        def proj_load_w(w_d, mc):
            w = wpool.tile([128, NCH, 128], F16, tag="w")
            src = w_d.rearrange("(kc p) m -> p kc m", p=128)
            nc.sync.dma_start(w[:], src[:, :, mc * 128:(mc + 1) * 128])
            return w

        w_first = proj_load_w(wq_d, 0)
        xt = big.tile([128, NCH, R], F16)
        for kc in range(NCH):
            nc.sync.dma_start(xt[:, kc, 0:512], xT_d[kc * 128:(kc + 1) * 128, 0:512])
        for kc in range(NCH):
            nc.sync.dma_start(xt[:, kc, 512:R], xT_d[kc * 128:(kc + 1) * 128, 512:R])
        c2 = consts.tile([128, R], F16)
        nc.sync.dma_start(c2[:], c2_d[:])
        s2 = consts.tile([128, R], F16)
        nc.sync.dma_start(s2[:], s2_d[:])

        qT = big.tile([128, NCH, R], F16)
        kT = big.tile([128, NCH, R], F16)
        # v holds 16 heads x (64 dims + a ones column) per row-chunk: the
        # ones column makes each AV matmul also emit the softmax row-sums
        # (output row 64) for free.
        v = big.tile([128, NCH, H * 65], BF16)
        ones16f = consts.tile([128, 16], F32)
        nc.vector.memset(ones16f[:], 1.0)
        # sg doubles as y: the attention tail multiplies the gate in-place
        # (av*rr*sg), and the out projection consumes it. fp16: it feeds
        # the fp16 out-projection matmul.
        sg = big.tile([128, NCH, R], F16)

        # ---- one projection output chunk: 8-matmul psum group + drain
        def mm_group(w, nh):
            ps = big_ps.tile([128, 512], F32, tag="big")
            for kc in range(NCH):
                nc.tensor.matmul(
                    ps[:],
                    w[:, kc, :],
                    xt[:, kc, nh * 512:(nh + 1) * 512],
                    start=(kc == 0),
                    stop=(kc == NCH - 1),
                )
            return ps

        def proj_chunk_gate(w, mc):
            # raw-gate tanh(g/2); finished to sigmoid by one fused DVE op
            for nh in range(2):
                ps = mm_group(w, nh)
                dstsl = sg[:, mc, nh * 512:(nh + 1) * 512]
                nc.scalar.activation(dstsl, ps[:], AF.Tanh, scale=0.5)
            # sigmoid(g) = 0.5*tanh(g/2) + 0.5, fp16 in-place at 2x DVE rate
            nc.vector.tensor_scalar(
                sg[:, mc, :], sg[:, mc, :], 0.5, 0.5, ALU.mult, ALU.add)

        def proj_chunk_rope(w, dst, mc):
            # RoPE: the rotate (partition swap d <-> d+-32 within each head)
            # rides on the otherwise-idle DMA engines as 4 partition-shifted
            # SBUF->SBUF copies of the raw projection; sign lives in sin2.
            ps0 = mm_group(w, 0)
            ps1 = mm_group(w, 1)
            raw = rawp.tile([128, R], F16, tag="raw")
            nc.scalar.activation(raw[:, 0:512], ps0[:], AF.Copy)
            nc.scalar.activation(raw[:, 512:1024], ps1[:], AF.Copy)
            t = ropet.tile([128, R], F16, tag="t")
            for h2 in (0, 64):
                nc.sync.dma_start(t[h2:h2 + 32, :], raw[h2 + 32:h2 + 64, :])
                nc.sync.dma_start(t[h2 + 32:h2 + 64, :], raw[h2:h2 + 32, :])
            dsl = dst[:, mc, :]
            nc.vector.tensor_mul(dsl, raw[:], c2[:])
            nc.vector.tensor_mul(t[:], t[:], s2[:])
            nc.vector.tensor_add(dsl, dsl, t[:])

        def attn_front(b, c):
            est = []
            for hi in range(2):
                pb = 64 * hi
                sps = s_ps_p.tile([128, 512], F32, tag="s")
                for kph in range(2):
                    nc.tensor.matmul(
                        sps[:, kph * 256:(kph + 1) * 256],
                        kT[pb:pb + 64, c,
                           b * 256 + kph * 128:b * 256 + (kph + 1) * 128],
                        qT[pb:pb + 64, c, b * 256:(b + 1) * 256],
                        start=True, stop=True,
                    )
                es = espool.tile([128, 512], BF16, tag="es")
                nc.scalar.activation(es[:], sps[:], AF.Exp,
                                     scale=float(SCALE))
                est.append(es)
            return (b, c, est)

        def attn_mid(st):
            b, c, est = st
            # both heads' AV share one psum bank: hi=0 in cols 0:256,
            # hi=1 in cols 256:512; row 0 = softmax row-sums (ones-led V)
            av = av_ps_p.tile([65, 512], F32, tag="av")
            for hi in range(2):
                es = est[hi]
                h = 2 * c + hi
                for kph in range(2):
                    nc.tensor.matmul(
                        av[:, hi * 256:(hi + 1) * 256],
                        v[:, 2 * b + kph, h * 65:(h + 1) * 65],
                        es[:, kph * 256:(kph + 1) * 256],
                        start=(kph == 0), stop=(kph == 1),
                    )
            # The DVE custom reciprocal misreads PSUM at base partition 64
            # on HW, so stage the sums row through SBUF first (ScalarE
            # handles base-64 PSUM reads fine; DVE is the loaded engine
            # in the tail chain).
            sumst = smalls.tile([1, 512], F32, tag="sumst")
            nc.scalar.activation(sumst[:], av[64:65, :], AF.Copy)
            recipf = smalls.tile([1, 512], F32, tag="recipf")
            nc.vector.reciprocal_approx_fast(recipf[:], sumst[:])
            # replicate recip over the 64 head dims on the idle Pool
            # engine. The Q7 broadcast maps DSP core j to partitions
            # [16j, 16j+16) with an absolute-partition mask, so the
            # destination MUST start at partition 0 -- use two base-0
            # tiles (one per head) instead of one [128, 256] tile.
            rrsb = rrsbp.tile([64, 512], F32, tag="rrsb")
            nc.gpsimd.partition_broadcast(rrsb[:], recipf[0:1, :])
            return (b, c, av, rrsb)

        def attn_tail(st):
            b, c, av, rrsb = st
            # y = av * rr * sg, written in-place over sg
            yt = ytp.tile([128, 256], F16, tag="yt")
            for hi in range(2):
                ysl_p = slice(64 * hi, 64 * hi + 64)
                nc.vector.tensor_mul(yt[ysl_p, :],
                                     av[0:64, hi * 256:(hi + 1) * 256],
                                     rrsb[0:64, hi * 256:(hi + 1) * 256])
                ysl = sg[ysl_p, c, b * 256:(b + 1) * 256]
                nc.vector.tensor_mul(ysl, ysl, yt[ysl_p, :])

        for rc in range(NCH):
            vh = v[:, rc, :].rearrange("p (h t) -> p h t", t=65)
            nc.scalar.activation(vh[:, :, 64], ones16f[:], AF.Copy)

        # ---- fused main loop: per chunk c, emit the q/qrot/k/krot
        # projections for chunk c, the v quarter (every other c), the gate
        # chunk, then the (pipelined) attention iterations for chunk c.
        # The projection matmuls keep PE dense while attention's serial
        # ACT/DVE chains (exp -> sums -> recip -> rr) drain, which also
        # keeps the HAM clock-gate warm.
        p1 = p2 = None
        for c in range(NCH):
            w = w_first if c == 0 else proj_load_w(wq_d, c)
            proj_chunk_rope(w, qT, c)
            w = proj_load_w(wk_d, c)
            proj_chunk_rope(w, kT, c)
            if c % 4 == 0:
                nv = c // 4
                wvb = wvpool.tile([128, NCH, 512], F16, tag="wv")
                for kc in range(NCH):
                    nc.sync.dma_start(
                        wvb[:, kc, :],
                        wv_d[kc * 128:(kc + 1) * 128,
                             nv * 512:(nv + 1) * 512])
                for rc in range(NCH):
                    ps = big_ps.tile([128, 512], F32, tag="big")
                    for kc in range(NCH):
                        nc.tensor.matmul(
                            ps[:],
                            xt[:, kc, rc * 128:(rc + 1) * 128],
                            wvb[:, kc, :],
                            start=(kc == 0),
                            stop=(kc == NCH - 1),
                        )
                    vh = v[:, rc, :].rearrange("p (h t) -> p h t", t=65)
                    nc.vector.tensor_copy(
                        vh[:, 8 * nv:8 * nv + 8, 0:64],
                        ps[:].rearrange("p (h d) -> p h d", d=64))
            w = proj_load_w(wg_d, c)
            proj_chunk_gate(w, c)
            for b in range(NB):
                cur = attn_front(b, c)
                if p1 is not None:
                    m = attn_mid(p1)
                    if p2 is not None:
                        attn_tail(p2)
                    p2 = m
                p1 = cur
        m = attn_mid(p1)
        attn_tail(p2)
        attn_tail(m)

        # ---- output projection (transposed): outT[of, r] = Wo.T @ y
        for oc in range(NCH):
            w = proj_load_w(wo_d, oc)
            for nh in range(2):
                ps = big_ps.tile([128, 512], F32, tag="big")
                for yc in range(NCH):
                    nc.tensor.matmul(
                        ps[:],
                        w[:, yc, :],
                        sg[:, yc, nh * 512:(nh + 1) * 512],
                        start=(yc == 0),
                        stop=(yc == NCH - 1),
                    )
                osb = opool.tile([128, 512], F32, tag="o")
                nc.scalar.activation(osb[:], ps[:], AF.Copy)
                nc.sync.dma_start(
                    outT_d[oc * 128:(oc + 1) * 128,
                           nh * 512:(nh + 1) * 512], osb[:])


def _build_nc():
    import concourse.bacc as bacc
    import concourse.mybir as mybir
    import concourse.tile as tile

    F32 = mybir.dt.float32
    F16 = mybir.dt.float16
    nc = bacc.Bacc("TRN2", target_bir_lowering=False, debug=False)
    names_in = ["xT", "wq", "wk", "wv", "wg", "wo", "cos2", "sin2"]
    shapes_in = [[E, R], [E, E], [E, E], [E, E], [E, E], [E, E],
                 [128, R], [128, R]]
    dts_in = [F16, F16, F16, F16, F16, F16, F16, F16]
    ins = [
        nc.dram_tensor(n, s, dt, kind="ExternalInput").ap()
        for n, s, dt in zip(names_in, shapes_in, dts_in)
    ]
    outT = nc.dram_tensor("outT", [E, R], F32, kind="ExternalOutput").ap()
    with tile.TileContext(nc) as tc:
        emit(tc, [outT], ins)
    nc.compile()
    return nc


_NC_CACHE = {}


def host_prep(x, Wq, Wk, Wv, Wg, Wo, cos, sin):
    """Build the 8 per-core input maps."""
    x_flat = np.ascontiguousarray(x.reshape(B * S, E), dtype=np.float32)
    Wq = np.ascontiguousarray(Wq, dtype=np.float16)
    Wk = np.ascontiguousarray(Wk, dtype=np.float16)
    Wv = np.ascontiguousarray(Wv, dtype=np.float16)
    Wg = np.ascontiguousarray(Wg, dtype=np.float16)
    Wo = np.ascontiguousarray(Wo, dtype=np.float16)
    cos = np.asarray(cos, dtype=np.float32)
    sin = np.asarray(sin, dtype=np.float32)
    sign = np.where(np.arange(D) < D // 2, -1.0, 1.0).astype(np.float32)

    in_maps = []
    for cix in range(NCORES):
        rows = slice(cix * R, (cix + 1) * R)
        xT = np.ascontiguousarray(x_flat[rows].T.astype(np.float16))
        seq = (cix * R + np.arange(R)) % S
        cS = cos[seq]            # [R, D]
        sS = sin[seq] * sign     # [R, D] signed
        c2 = np.ascontiguousarray(np.tile(cS.T, (2, 1)).astype(np.float16))
        s2 = np.ascontiguousarray(np.tile(sS.T, (2, 1)).astype(np.float16))
        in_maps.append({
            "xT": xT, "wq": Wq, "wk": Wk, "wv": Wv, "wg": Wg, "wo": Wo,
            "cos2": c2, "sin2": s2,
        })
    return in_maps


def kernel_traced(x, Wq, Wk, Wv, Wg, Wo, cos, sin, block_size, trace=False,
                  **run_kwargs):
    assert int(block_size) == BLK
    from concourse import bass_utils

    if "nc" not in _NC_CACHE:
        _NC_CACHE["nc"] = _build_nc()
    nc = _NC_CACHE["nc"]

    in_maps = host_prep(x, Wq, Wk, Wv, Wg, Wo, cos, sin)
    res = bass_utils.run_bass_kernel_spmd(
        nc, in_maps, core_ids=list(range(NCORES)), trace=trace, **run_kwargs)
    out_flat = np.empty((B * S, E), dtype=np.float32)
    for cix in range(NCORES):
        out_flat[cix * R:(cix + 1) * R] = res.results[cix]["outT"].T
    return out_flat.reshape(B, S, E), res


def kernel(x, Wq, Wk, Wv, Wg, Wo, cos, sin, block_size):
    return kernel_traced(x, Wq, Wk, Wv, Wg, Wo, cos, sin, block_size)[0]
